# revision 7
# baseline (speedup 1.0000x reference)
"""GNN message passing (src_mul_edge + segment_sum) on 8 Trainium2 cores. v6.

out[n] = sum_{e : dst[e]==n} e_att[e] * src_emb[src[e]]

PE scatter-matmul design:
  * src_emb rows cast to fp16; consecutive row PAIRS form 256-byte tokens in
    DRAM ([25088, 128] fp16); token ids fit int16 (single index window).
  * Nodes bin-packed into GROUPS: <=32 nodes, total degree <= 512 slots
    (4 blocks of 128). Groups uniform across cores -> one shared program.
  * Edge slots: per group, edges sorted by token; pad slots use token 0 with
    A = 0. dma_gather(transpose=False) -> msg[slot%128, block, 0:128] fp16,
    round-robin over all 4 SWDGE queues (8 Q7 descriptor cores).
  * Per 128-slot block: two matmuls on the (otherwise idle) tensor engine:
      psum[32 nodes, 64] += A_ev[128,32].T @ msg[:,b,0:64]
                          + A_od[128,32].T @ msg[:,b,64:128]
    where A_ev/A_od hold att for even/odd-half edges (0 elsewhere) and
    PSUM accumulates over the group's 4 blocks.
  * Supertile = 4 groups = one PSUM tile [128, 64] fp32; DVE evacuates to
    SBUF, single DMA to DRAM out. Host unpermutes rows.
"""

import numpy as np

N_SRC = 50000
N_DST = 50000
D = 64
N_CORES = 8
P = 128
NPAIR = 25088
GROUP_SLOTS = 512          # 4 blocks of 128
GROUP_NODES = 32           # PSUM strip width
CHUNK_BLOCKS = 32          # gather chunk = 32 blocks = 4096 slots
PRIME_BLOCKS = 4           # first 4 chunks are 4 blocks each

_cache: dict = {}

TRACE = False
TRACE_DIR = None
LAST_EXEC_NS = None


def _wrap_idx(idx_flat):
    w = idx_flat.reshape(-1, 16).T
    return np.tile(w, (8, 1))


def _binpack(deg):
    """Pack nodes into groups: <=GROUP_NODES nodes, sum(deg) <= GROUP_SLOTS.
    Picks the available degree closest to the remaining per-node target so the
    degree mix stays balanced through the tail. Returns list of node-id lists."""
    maxd = int(deg.max())
    order = np.argsort(-deg, kind="stable")
    nodes_by_deg = [[] for _ in range(maxd + 1)]
    for n in order:
        d = deg[n]
        if d > 0:
            nodes_by_deg[d].append(int(n))
    ptr = [0] * (maxd + 1)
    avail = [len(nodes_by_deg[d]) - ptr[d] for d in range(maxd + 1)]
    remaining = sum(avail[1:])
    groups = []
    while remaining > 0:
        cap = GROUP_SLOTS
        members = []
        while len(members) < GROUP_NODES and cap > 0:
            tgt = cap / (GROUP_NODES - len(members))
            best = -1
            bestdist = None
            d = min(cap, maxd)
            while d >= 1:
                if avail[d] > 0:
                    dist = abs(d - tgt)
                    if bestdist is None or dist < bestdist:
                        bestdist = dist
                        best = d
                    elif d < tgt and dist > bestdist:
                        break
                d -= 1
            if best < 0:
                break
            members.append(nodes_by_deg[best][ptr[best]])
            ptr[best] += 1
            avail[best] -= 1
            remaining -= 1
            cap -= best
        groups.append(members)
    return groups


def _plan(dst_idx, tok, half, att):
    deg = np.bincount(dst_idx, minlength=N_DST)
    groups = _binpack(deg)
    ng = len(groups)
    # per-core group count: multiple of 3 (supertile = 3 groups / 96 psum rows)
    G = -(-ng // N_CORES)
    G = -(-G // 3) * 3
    ng_pad = G * N_CORES
    NB = G * 4                      # blocks per core
    NS = NB * P                     # slots per core

    # group id (global, 0..ng_pad), col within group for every node
    grp_of = np.full(N_DST, -1, dtype=np.int64)
    col_of = np.full(N_DST, -1, dtype=np.int64)
    node_at = np.full((ng_pad, GROUP_NODES), -1, dtype=np.int64)
    for g, members in enumerate(groups):
        m = np.asarray(members, dtype=np.int64)
        grp_of[m] = g
        col_of[m] = np.arange(len(m))
        node_at[g, : len(m)] = m

    E = len(dst_idx)
    g_e = grp_of[dst_idx]
    assert (g_e >= 0).all()
    eorder = np.lexsort((tok, g_e))
    g_s = g_e[eorder]
    # rank of each edge within its group
    gstart = np.searchsorted(g_s, np.arange(ng + 1))
    rank = np.arange(E) - gstart[g_s]
    slot_global = g_s * GROUP_SLOTS + rank      # 0 .. ng*512
    core_e = slot_global // (G * GROUP_SLOTS)
    slot_e = slot_global % (G * GROUP_SLOTS)

    # per-core tables
    idx2 = np.zeros((N_CORES, NS), dtype=np.int16)           # token per slot
    a3 = np.zeros((N_CORES, NS, 2 * GROUP_NODES), dtype=np.float16)
    col_e = col_of[dst_idx][eorder]
    half_e = half[eorder]
    idx2[core_e, slot_e] = tok[eorder]
    a3[core_e, slot_e, col_e + GROUP_NODES * half_e] = att[eorder]

    # chunk schedule (blocks): 4 priming chunks, then CHUNK_BLOCKS
    chunks = []
    b0 = 0
    while b0 < NB:
        nb = PRIME_BLOCKS if len(chunks) < 4 else CHUNK_BLOCKS
        nb = min(nb, NB - b0)
        chunks.append((b0, nb))
        b0 += nb

    return {
        "NB": NB,
        "G": G,
        "chunks": tuple(chunks),
        "idx2": idx2,
        "a3": a3,
        "node_at": node_at,
        "pad_frac": 1.0 - E / (ng_pad * GROUP_SLOTS),
    }


def _build_nc(NB, chunks):
    import concourse.bacc as bacc
    import concourse.mybir as mybir
    from concourse.tile import TileContext
    from concourse.library_config import mlp

    NS = NB * P
    nsuper = NB // 12

    nc = bacc.Bacc(
        "TRN2", target_bir_lowering=False, debug=False, num_swdge_queues=4
    )
    embP = nc.dram_tensor("embP", [NPAIR, P], mybir.dt.float16, kind="ExternalInput")
    idxT = nc.dram_tensor("idxT", [P, NS // 16], mybir.dt.int16, kind="ExternalInput")
    atab = nc.dram_tensor("atab", [P, NB * 64], mybir.dt.float16, kind="ExternalInput")
    out = nc.dram_tensor("out", [nsuper * 96, D], mybir.dt.float32, kind="ExternalOutput")

    with TileContext(nc) as tc:
        nc.gpsimd.load_library(mlp)
        with (
            tc.tile_pool(name="tbl", bufs=1) as tbl,
            tc.tile_pool(name="msg", bufs=8) as msgp,
            tc.tile_pool(name="apool", bufs=4) as apool,
            tc.tile_pool(name="psum", bufs=8, space="PSUM") as psump,
            tc.tile_pool(name="stg", bufs=4) as stgp,
        ):
            # two-stage idx load: head slice unblocks the first gathers
            head_blocks = sum(nb for _, nb in chunks[:5])
            head_cols = head_blocks * 8
            tail_cols = NS // 16 - head_cols
            idx_a = tbl.tile([P, head_cols], mybir.dt.int16, tag="idxa")
            nc.sync.dma_start(idx_a[:], idxT[:, :head_cols])
            if tail_cols > 0:
                idx_b = tbl.tile([P, tail_cols], mybir.dt.int16, tag="idxb")
                nc.sync.dma_start(idx_b[:], idxT[:, head_cols:])

            psum_tiles = {}
            stage_tiles = {}
            for ci, (b0, nb) in enumerate(chunks):
                q = ci % 4
                c_lo, c_hi = b0 * 8, (b0 + nb) * 8
                if c_hi <= head_cols:
                    iap = idx_a[:, c_lo:c_hi]
                else:
                    iap = idx_b[:, c_lo - head_cols : c_hi - head_cols]
                nidx = nb * P
                msg = msgp.tile([P, CHUNK_BLOCKS, P], mybir.dt.float16, tag="m")
                nc.gpsimd.dma_gather(
                    msg[:, :nb, :], embP[:, :],
                    iap, nidx, nidx, P,
                    transpose=False, single_packet=False, queue_num=q,
                )
                a_t = apool.tile([P, CHUNK_BLOCKS * 64], mybir.dt.float16, tag="a")
                nc.scalar.dma_start(a_t[:, : nb * 64], atab[:, b0 * 64 : (b0 + nb) * 64])

                for j in range(nb):
                    b = b0 + j
                    g = b // 4                   # group id
                    st = b // 12
                    gl = g % 3                   # group within supertile
                    if g not in psum_tiles:
                        # [64, 128]: quadrant trick — rows 0:32 even part
                        # (valid cols 0:64), rows 32:64 odd part (cols 64:128)
                        psum_tiles[g] = psump.tile(
                            [64, P], mybir.dt.float32, tag="ps", name=f"ps{g}"
                        )
                    ps = psum_tiles[g]
                    nc.tensor.matmul(
                        ps[:, :], a_t[:, j * 64 : j * 64 + 64], msg[:, j, :],
                        start=(b % 4 == 0), stop=(b % 4 == 3),
                    )
                    if b % 4 == 3:
                        if gl == 0:
                            stage_tiles[st] = stgp.tile(
                                [96, D], mybir.dt.float32, tag="st", name=f"st{st}"
                            )
                        stage = stage_tiles[st]
                        nc.scalar.copy(
                            stage[32 * gl : 32 * gl + 32, :], ps[0:32, 0:D]
                        )
                        nc.vector.tensor_tensor(
                            stage[32 * gl : 32 * gl + 32, :],
                            stage[32 * gl : 32 * gl + 32, :],
                            ps[32:64, D : 2 * D],
                            mybir.AluOpType.add,
                        )
                        del psum_tiles[g]
                        if gl == 2:
                            nc.sync.dma_start(
                                out[st * 96 : (st + 1) * 96, :], stage[:, :]
                            )
                            del stage_tiles[st]
    nc.compile()
    return nc


def plan_and_build(src_idx, dst_idx, e_att):
    src_idx = np.asarray(src_idx, dtype=np.int64)
    dst_idx = np.asarray(dst_idx, dtype=np.int64)
    att_flat = np.asarray(e_att, dtype=np.float16).reshape(-1)
    tok = (src_idx // 2).astype(np.int16)
    half = (src_idx & 1).astype(np.int64)
    return _plan(dst_idx, tok, half, att_flat)


def kernel(src_emb, e_att, src_idx, dst_idx):
    from concourse.bass_utils import run_bass_kernel_spmd

    src_emb = np.asarray(src_emb, dtype=np.float32)
    pl = plan_and_build(src_idx, dst_idx, e_att)

    key = (pl["NB"], pl["chunks"])
    if key not in _cache:
        _cache.clear()
        _cache[key] = _build_nc(pl["NB"], pl["chunks"])
    nc = _cache[key]

    embP = np.zeros((NPAIR * 2, D), dtype=np.float16)
    embP[:N_SRC] = src_emb.astype(np.float16)
    embP = np.ascontiguousarray(embP.reshape(NPAIR, P))

    NB = pl["NB"]
    in_maps = []
    for c in range(N_CORES):
        # atab layout: [128 (slot in block), NB*64] fp16
        at = np.ascontiguousarray(
            pl["a3"][c].reshape(NB, P, 64).transpose(1, 0, 2).reshape(P, NB * 64)
        )
        in_maps.append(
            {
                "embP": embP,
                "idxT": np.ascontiguousarray(_wrap_idx(pl["idx2"][c].reshape(-1))),
                "atab": at,
            }
        )
    kwargs = {}
    if TRACE:
        kwargs = {"trace": True, "tmpdir": TRACE_DIR}
    res = run_bass_kernel_spmd(nc, in_maps, core_ids=list(range(N_CORES)), **kwargs)
    global LAST_EXEC_NS
    LAST_EXEC_NS = res.exec_time_ns

    out_full = np.zeros((N_DST, D), dtype=np.float32)
    G = pl["G"]
    node_at = pl["node_at"]  # [ng_pad, 32]
    for c in range(N_CORES):
        ids = node_at[c * G : (c + 1) * G].reshape(-1)   # supertile-row order
        valid = ids >= 0
        out_full[ids[valid]] = res.results[c]["out"][valid]
    return out_full


# revision 8
# speedup vs baseline: 1.0113x; 1.0113x over previous
"""GNN message passing (src_mul_edge + segment_sum) on 8 Trainium2 cores. v6.

out[n] = sum_{e : dst[e]==n} e_att[e] * src_emb[src[e]]

PE scatter-matmul design:
  * src_emb rows cast to fp16; consecutive row PAIRS form 256-byte tokens in
    DRAM ([25088, 128] fp16); token ids fit int16 (single index window).
  * Nodes bin-packed into GROUPS: <=32 nodes, total degree <= 512 slots
    (4 blocks of 128). Groups uniform across cores -> one shared program.
  * Edge slots: per group, edges sorted by token; pad slots use token 0 with
    A = 0. dma_gather(transpose=False) -> msg[slot%128, block, 0:128] fp16,
    round-robin over all 4 SWDGE queues (8 Q7 descriptor cores).
  * Per 128-slot block: two matmuls on the (otherwise idle) tensor engine:
      psum[32 nodes, 64] += A_ev[128,32].T @ msg[:,b,0:64]
                          + A_od[128,32].T @ msg[:,b,64:128]
    where A_ev/A_od hold att for even/odd-half edges (0 elsewhere) and
    PSUM accumulates over the group's 4 blocks.
  * Supertile = 4 groups = one PSUM tile [128, 64] fp32; DVE evacuates to
    SBUF, single DMA to DRAM out. Host unpermutes rows.
"""

import numpy as np

N_SRC = 50000
N_DST = 50000
D = 64
N_CORES = 8
P = 128
NPAIR = 25088
GROUP_SLOTS = 512          # 4 blocks of 128
GROUP_NODES = 32           # PSUM strip width
CHUNK_BLOCKS = 32          # gather chunk = 32 blocks = 4096 slots
PRIME_BLOCKS = 4           # first 4 chunks are 4 blocks each

_cache: dict = {}

TRACE = False
TRACE_DIR = None
LAST_EXEC_NS = None


def _wrap_idx(idx_flat):
    w = idx_flat.reshape(-1, 16).T
    return np.tile(w, (8, 1))


def _binpack(deg):
    """Pack nodes into groups: <=GROUP_NODES nodes, sum(deg) <= GROUP_SLOTS.
    Picks the available degree closest to the remaining per-node target so the
    degree mix stays balanced through the tail. Returns list of node-id lists."""
    maxd = int(deg.max())
    order = np.argsort(-deg, kind="stable")
    nodes_by_deg = [[] for _ in range(maxd + 1)]
    for n in order:
        d = deg[n]
        if d > 0:
            nodes_by_deg[d].append(int(n))
    ptr = [0] * (maxd + 1)
    avail = [len(nodes_by_deg[d]) - ptr[d] for d in range(maxd + 1)]
    remaining = sum(avail[1:])
    groups = []
    while remaining > 0:
        cap = GROUP_SLOTS
        members = []
        while len(members) < GROUP_NODES and cap > 0:
            tgt = cap / (GROUP_NODES - len(members))
            best = -1
            bestdist = None
            d = min(cap, maxd)
            while d >= 1:
                if avail[d] > 0:
                    dist = abs(d - tgt)
                    if bestdist is None or dist < bestdist:
                        bestdist = dist
                        best = d
                    elif d < tgt and dist > bestdist:
                        break
                d -= 1
            if best < 0:
                break
            members.append(nodes_by_deg[best][ptr[best]])
            ptr[best] += 1
            avail[best] -= 1
            remaining -= 1
            cap -= best
        groups.append(members)
    return groups


def _plan(dst_idx, tok, half, att):
    deg = np.bincount(dst_idx, minlength=N_DST)
    groups = _binpack(deg)
    ng = len(groups)
    # per-core group count: multiple of 3 (supertile = 3 groups / 96 psum rows)
    G = -(-ng // N_CORES)
    G = -(-G // 3) * 3
    ng_pad = G * N_CORES
    NB = G * 4                      # blocks per core
    NS = NB * P                     # slots per core

    # group id (global, 0..ng_pad), col within group for every node
    grp_of = np.full(N_DST, -1, dtype=np.int64)
    col_of = np.full(N_DST, -1, dtype=np.int64)
    node_at = np.full((ng_pad, GROUP_NODES), -1, dtype=np.int64)
    for g, members in enumerate(groups):
        m = np.asarray(members, dtype=np.int64)
        grp_of[m] = g
        col_of[m] = np.arange(len(m))
        node_at[g, : len(m)] = m

    E = len(dst_idx)
    g_e = grp_of[dst_idx]
    assert (g_e >= 0).all()
    eorder = np.lexsort((tok, g_e))
    g_s = g_e[eorder]
    # rank of each edge within its group
    gstart = np.searchsorted(g_s, np.arange(ng + 1))
    rank = np.arange(E) - gstart[g_s]
    slot_global = g_s * GROUP_SLOTS + rank      # 0 .. ng*512
    core_e = slot_global // (G * GROUP_SLOTS)
    slot_e = slot_global % (G * GROUP_SLOTS)

    # per-core tables
    idx2 = np.zeros((N_CORES, NS), dtype=np.int16)           # token per slot
    a3 = np.zeros((N_CORES, NS, 2 * GROUP_NODES), dtype=np.float16)
    col_e = col_of[dst_idx][eorder]
    half_e = half[eorder]
    idx2[core_e, slot_e] = tok[eorder]
    a3[core_e, slot_e, col_e + GROUP_NODES * half_e] = att[eorder]

    # chunk schedule (blocks): 4 priming chunks, then CHUNK_BLOCKS
    chunks = []
    b0 = 0
    while b0 < NB:
        nb = PRIME_BLOCKS if len(chunks) < 4 else CHUNK_BLOCKS
        nb = min(nb, NB - b0)
        chunks.append((b0, nb))
        b0 += nb

    return {
        "NB": NB,
        "G": G,
        "chunks": tuple(chunks),
        "idx2": idx2,
        "a3": a3,
        "node_at": node_at,
        "pad_frac": 1.0 - E / (ng_pad * GROUP_SLOTS),
    }


def _build_nc(NB, chunks):
    import concourse.bacc as bacc
    import concourse.mybir as mybir
    from concourse.tile import TileContext
    from concourse.library_config import mlp

    NS = NB * P
    nsuper = NB // 12

    nc = bacc.Bacc(
        "TRN2", target_bir_lowering=False, debug=False, num_swdge_queues=4,
        dynamic_dma_scratch_size=65536,
    )
    embP = nc.dram_tensor("embP", [NPAIR, P], mybir.dt.float16, kind="ExternalInput")
    idxT = nc.dram_tensor("idxT", [P, NS // 16], mybir.dt.int16, kind="ExternalInput")
    atab = nc.dram_tensor("atab", [P, NB * 64], mybir.dt.float16, kind="ExternalInput")
    out = nc.dram_tensor("out", [nsuper * 96, D], mybir.dt.float32, kind="ExternalOutput")

    with TileContext(nc) as tc:
        nc.gpsimd.load_library(mlp)
        with (
            tc.tile_pool(name="tbl", bufs=1) as tbl,
            tc.tile_pool(name="msg", bufs=8) as msgp,
            tc.tile_pool(name="apool", bufs=4) as apool,
            tc.tile_pool(name="psum", bufs=8, space="PSUM") as psump,
            tc.tile_pool(name="stg", bufs=4) as stgp,
        ):
            # two-stage idx load: head slice unblocks the first gathers
            head_blocks = sum(nb for _, nb in chunks[:5])
            head_cols = head_blocks * 8
            tail_cols = NS // 16 - head_cols
            idx_a = tbl.tile([P, head_cols], mybir.dt.int16, tag="idxa")
            nc.sync.dma_start(idx_a[:], idxT[:, :head_cols])
            if tail_cols > 0:
                idx_b = tbl.tile([P, tail_cols], mybir.dt.int16, tag="idxb")
                nc.sync.dma_start(idx_b[:], idxT[:, head_cols:])

            psum_tiles = {}
            stage_tiles = {}
            for ci, (b0, nb) in enumerate(chunks):
                q = ci % 4
                c_lo, c_hi = b0 * 8, (b0 + nb) * 8
                if c_hi <= head_cols:
                    iap = idx_a[:, c_lo:c_hi]
                else:
                    iap = idx_b[:, c_lo - head_cols : c_hi - head_cols]
                nidx = nb * P
                msg = msgp.tile([P, CHUNK_BLOCKS, P], mybir.dt.float16, tag="m")
                nc.gpsimd.dma_gather(
                    msg[:, :nb, :], embP[:, :],
                    iap, nidx, nidx, P,
                    transpose=False, single_packet=False, queue_num=q,
                )
                a_t = apool.tile([P, CHUNK_BLOCKS * 64], mybir.dt.float16, tag="a")
                nc.scalar.dma_start(a_t[:, : nb * 64], atab[:, b0 * 64 : (b0 + nb) * 64])

                for j in range(nb):
                    b = b0 + j
                    g = b // 4                   # group id
                    st = b // 12
                    gl = g % 3                   # group within supertile
                    if g not in psum_tiles:
                        # [64, 128]: quadrant trick — rows 0:32 even part
                        # (valid cols 0:64), rows 32:64 odd part (cols 64:128)
                        psum_tiles[g] = psump.tile(
                            [64, P], mybir.dt.float32, tag="ps", name=f"ps{g}"
                        )
                    ps = psum_tiles[g]
                    nc.tensor.matmul(
                        ps[:, :], a_t[:, j * 64 : j * 64 + 64], msg[:, j, :],
                        start=(b % 4 == 0), stop=(b % 4 == 3),
                    )
                    if b % 4 == 3:
                        if gl == 0:
                            stage_tiles[st] = stgp.tile(
                                [96, D], mybir.dt.float32, tag="st", name=f"st{st}"
                            )
                        stage = stage_tiles[st]
                        nc.scalar.copy(
                            stage[32 * gl : 32 * gl + 32, :], ps[0:32, 0:D]
                        )
                        nc.vector.tensor_tensor(
                            stage[32 * gl : 32 * gl + 32, :],
                            stage[32 * gl : 32 * gl + 32, :],
                            ps[32:64, D : 2 * D],
                            mybir.AluOpType.add,
                        )
                        del psum_tiles[g]
                        if gl == 2:
                            nc.sync.dma_start(
                                out[st * 96 : (st + 1) * 96, :], stage[:, :]
                            )
                            del stage_tiles[st]
    nc.compile()
    return nc


def plan_and_build(src_idx, dst_idx, e_att):
    src_idx = np.asarray(src_idx, dtype=np.int64)
    dst_idx = np.asarray(dst_idx, dtype=np.int64)
    att_flat = np.asarray(e_att, dtype=np.float16).reshape(-1)
    tok = (src_idx // 2).astype(np.int16)
    half = (src_idx & 1).astype(np.int64)
    return _plan(dst_idx, tok, half, att_flat)


def kernel(src_emb, e_att, src_idx, dst_idx):
    from concourse.bass_utils import run_bass_kernel_spmd

    src_emb = np.asarray(src_emb, dtype=np.float32)
    pl = plan_and_build(src_idx, dst_idx, e_att)

    key = (pl["NB"], pl["chunks"])
    if key not in _cache:
        _cache.clear()
        _cache[key] = _build_nc(pl["NB"], pl["chunks"])
    nc = _cache[key]

    embP = np.zeros((NPAIR * 2, D), dtype=np.float16)
    embP[:N_SRC] = src_emb.astype(np.float16)
    embP = np.ascontiguousarray(embP.reshape(NPAIR, P))

    NB = pl["NB"]
    in_maps = []
    for c in range(N_CORES):
        # atab layout: [128 (slot in block), NB*64] fp16
        at = np.ascontiguousarray(
            pl["a3"][c].reshape(NB, P, 64).transpose(1, 0, 2).reshape(P, NB * 64)
        )
        in_maps.append(
            {
                "embP": embP,
                "idxT": np.ascontiguousarray(_wrap_idx(pl["idx2"][c].reshape(-1))),
                "atab": at,
            }
        )
    kwargs = {}
    if TRACE:
        kwargs = {"trace": True, "tmpdir": TRACE_DIR}
    res = run_bass_kernel_spmd(nc, in_maps, core_ids=list(range(N_CORES)), **kwargs)
    global LAST_EXEC_NS
    LAST_EXEC_NS = res.exec_time_ns

    out_full = np.zeros((N_DST, D), dtype=np.float32)
    G = pl["G"]
    node_at = pl["node_at"]  # [ng_pad, 32]
    for c in range(N_CORES):
        ids = node_at[c * G : (c + 1) * G].reshape(-1)   # supertile-row order
        valid = ids >= 0
        out_full[ids[valid]] = res.results[c]["out"][valid]
    return out_full


# revision 9
# speedup vs baseline: 1.0413x; 1.0297x over previous
"""GNN message passing (src_mul_edge + segment_sum) on 8 Trainium2 cores. v6.

out[n] = sum_{e : dst[e]==n} e_att[e] * src_emb[src[e]]

PE scatter-matmul design:
  * src_emb rows cast to fp16; consecutive row PAIRS form 256-byte tokens in
    DRAM ([25088, 128] fp16); token ids fit int16 (single index window).
  * Nodes bin-packed into GROUPS: <=32 nodes, total degree <= 512 slots
    (4 blocks of 128). Groups uniform across cores -> one shared program.
  * Edge slots: per group, edges sorted by token; pad slots use token 0 with
    A = 0. dma_gather(transpose=False) -> msg[slot%128, block, 0:128] fp16,
    round-robin over all 4 SWDGE queues (8 Q7 descriptor cores).
  * Per 128-slot block: two matmuls on the (otherwise idle) tensor engine:
      psum[32 nodes, 64] += A_ev[128,32].T @ msg[:,b,0:64]
                          + A_od[128,32].T @ msg[:,b,64:128]
    where A_ev/A_od hold att for even/odd-half edges (0 elsewhere) and
    PSUM accumulates over the group's 4 blocks.
  * Supertile = 4 groups = one PSUM tile [128, 64] fp32; DVE evacuates to
    SBUF, single DMA to DRAM out. Host unpermutes rows.
"""

import numpy as np

N_SRC = 50000
N_DST = 50000
D = 64
N_CORES = 8
P = 128
NPAIR = 25088
GROUP_SLOTS = 512          # 4 blocks of 128
GROUP_NODES = 32           # PSUM strip width
CHUNK_BLOCKS = 16          # gather chunk = 16 blocks = 2048 slots
PRIME_BLOCKS = 4           # first 4 chunks are 4 blocks each

_cache: dict = {}

TRACE = False
TRACE_DIR = None
LAST_EXEC_NS = None


def _wrap_idx(idx_flat):
    w = idx_flat.reshape(-1, 16).T
    return np.tile(w, (8, 1))


def _binpack(deg):
    """Pack nodes into groups: <=GROUP_NODES nodes, sum(deg) <= GROUP_SLOTS.
    Picks the available degree closest to the remaining per-node target so the
    degree mix stays balanced through the tail. Returns list of node-id lists."""
    maxd = int(deg.max())
    order = np.argsort(-deg, kind="stable")
    nodes_by_deg = [[] for _ in range(maxd + 1)]
    for n in order:
        d = deg[n]
        if d > 0:
            nodes_by_deg[d].append(int(n))
    ptr = [0] * (maxd + 1)
    avail = [len(nodes_by_deg[d]) - ptr[d] for d in range(maxd + 1)]
    remaining = sum(avail[1:])
    groups = []
    while remaining > 0:
        cap = GROUP_SLOTS
        members = []
        while len(members) < GROUP_NODES and cap > 0:
            tgt = cap / (GROUP_NODES - len(members))
            best = -1
            bestdist = None
            d = min(cap, maxd)
            while d >= 1:
                if avail[d] > 0:
                    dist = abs(d - tgt)
                    if bestdist is None or dist < bestdist:
                        bestdist = dist
                        best = d
                    elif d < tgt and dist > bestdist:
                        break
                d -= 1
            if best < 0:
                break
            members.append(nodes_by_deg[best][ptr[best]])
            ptr[best] += 1
            avail[best] -= 1
            remaining -= 1
            cap -= best
        groups.append(members)
    return groups


def _plan(dst_idx, tok, half, att):
    deg = np.bincount(dst_idx, minlength=N_DST)
    groups = _binpack(deg)
    ng = len(groups)
    # per-core group count: multiple of 3 (supertile = 3 groups / 96 psum rows)
    G = -(-ng // N_CORES)
    G = -(-G // 3) * 3
    ng_pad = G * N_CORES
    NB = G * 4                      # blocks per core
    NS = NB * P                     # slots per core

    # group id (global, 0..ng_pad), col within group for every node
    grp_of = np.full(N_DST, -1, dtype=np.int64)
    col_of = np.full(N_DST, -1, dtype=np.int64)
    node_at = np.full((ng_pad, GROUP_NODES), -1, dtype=np.int64)
    for g, members in enumerate(groups):
        m = np.asarray(members, dtype=np.int64)
        grp_of[m] = g
        col_of[m] = np.arange(len(m))
        node_at[g, : len(m)] = m

    E = len(dst_idx)
    g_e = grp_of[dst_idx]
    assert (g_e >= 0).all()
    eorder = np.lexsort((tok, g_e))
    g_s = g_e[eorder]
    # rank of each edge within its group
    gstart = np.searchsorted(g_s, np.arange(ng + 1))
    rank = np.arange(E) - gstart[g_s]
    slot_global = g_s * GROUP_SLOTS + rank      # 0 .. ng*512
    core_e = slot_global // (G * GROUP_SLOTS)
    slot_e = slot_global % (G * GROUP_SLOTS)

    # per-core tables
    idx2 = np.zeros((N_CORES, NS), dtype=np.int16)           # token per slot
    a3 = np.zeros((N_CORES, NS, 2 * GROUP_NODES), dtype=np.float16)
    col_e = col_of[dst_idx][eorder]
    half_e = half[eorder]
    idx2[core_e, slot_e] = tok[eorder]
    a3[core_e, slot_e, col_e + GROUP_NODES * half_e] = att[eorder]

    # chunk schedule (blocks): 4 priming chunks, then CHUNK_BLOCKS
    chunks = []
    b0 = 0
    while b0 < NB:
        nb = PRIME_BLOCKS if len(chunks) < 4 else CHUNK_BLOCKS
        nb = min(nb, NB - b0)
        chunks.append((b0, nb))
        b0 += nb

    return {
        "NB": NB,
        "G": G,
        "chunks": tuple(chunks),
        "idx2": idx2,
        "a3": a3,
        "node_at": node_at,
        "pad_frac": 1.0 - E / (ng_pad * GROUP_SLOTS),
    }


def _build_nc(NB, chunks):
    import concourse.bacc as bacc
    import concourse.mybir as mybir
    from concourse.tile import TileContext
    from concourse.library_config import mlp

    NS = NB * P
    nsuper = NB // 12

    nc = bacc.Bacc(
        "TRN2", target_bir_lowering=False, debug=False, num_swdge_queues=4,
        dynamic_dma_scratch_size=65536,
    )
    embP = nc.dram_tensor("embP", [NPAIR, P], mybir.dt.float16, kind="ExternalInput")
    idxT = nc.dram_tensor("idxT", [P, NS // 16], mybir.dt.int16, kind="ExternalInput")
    atab = nc.dram_tensor("atab", [P, NB * 64], mybir.dt.float16, kind="ExternalInput")
    out = nc.dram_tensor("out", [nsuper * 96, D], mybir.dt.float32, kind="ExternalOutput")

    with TileContext(nc) as tc:
        nc.gpsimd.load_library(mlp)
        with (
            tc.tile_pool(name="tbl", bufs=1) as tbl,
            tc.tile_pool(name="msg", bufs=12) as msgp,
            tc.tile_pool(name="apool", bufs=8) as apool,
            tc.tile_pool(name="psum", bufs=8, space="PSUM") as psump,
            tc.tile_pool(name="stg", bufs=6) as stgp,
        ):
            # two-stage idx load: head slice unblocks the first gathers
            head_blocks = sum(nb for _, nb in chunks[:5])
            head_cols = head_blocks * 8
            tail_cols = NS // 16 - head_cols
            idx_a = tbl.tile([P, head_cols], mybir.dt.int16, tag="idxa")
            nc.sync.dma_start(idx_a[:], idxT[:, :head_cols])
            if tail_cols > 0:
                idx_b = tbl.tile([P, tail_cols], mybir.dt.int16, tag="idxb")
                nc.sync.dma_start(idx_b[:], idxT[:, head_cols:])

            psum_tiles = {}
            stage_tiles = {}
            for ci, (b0, nb) in enumerate(chunks):
                q = ci % 4
                c_lo, c_hi = b0 * 8, (b0 + nb) * 8
                if c_hi <= head_cols:
                    iap = idx_a[:, c_lo:c_hi]
                else:
                    iap = idx_b[:, c_lo - head_cols : c_hi - head_cols]
                nidx = nb * P
                msg = msgp.tile([P, CHUNK_BLOCKS, P], mybir.dt.float16, tag="m")
                nc.gpsimd.dma_gather(
                    msg[:, :nb, :], embP[:, :],
                    iap, nidx, nidx, P,
                    transpose=False, single_packet=False, queue_num=q,
                )
                a_t = apool.tile([P, CHUNK_BLOCKS * 64], mybir.dt.float16, tag="a")
                nc.scalar.dma_start(a_t[:, : nb * 64], atab[:, b0 * 64 : (b0 + nb) * 64])

                for j in range(nb):
                    b = b0 + j
                    g = b // 4                   # group id
                    st = b // 12
                    gl = g % 3                   # group within supertile
                    if g not in psum_tiles:
                        # [64, 128]: quadrant trick — rows 0:32 even part
                        # (valid cols 0:64), rows 32:64 odd part (cols 64:128)
                        psum_tiles[g] = psump.tile(
                            [64, P], mybir.dt.float32, tag="ps", name=f"ps{g}"
                        )
                    ps = psum_tiles[g]
                    nc.tensor.matmul(
                        ps[:, :], a_t[:, j * 64 : j * 64 + 64], msg[:, j, :],
                        start=(b % 4 == 0), stop=(b % 4 == 3),
                    )
                    if b % 4 == 3:
                        if gl == 0:
                            stage_tiles[st] = stgp.tile(
                                [96, D], mybir.dt.float32, tag="st", name=f"st{st}"
                            )
                        stage = stage_tiles[st]
                        nc.scalar.copy(
                            stage[32 * gl : 32 * gl + 32, :], ps[0:32, 0:D]
                        )
                        nc.vector.tensor_tensor(
                            stage[32 * gl : 32 * gl + 32, :],
                            stage[32 * gl : 32 * gl + 32, :],
                            ps[32:64, D : 2 * D],
                            mybir.AluOpType.add,
                        )
                        del psum_tiles[g]
                        if gl == 2:
                            nc.sync.dma_start(
                                out[st * 96 : (st + 1) * 96, :], stage[:, :]
                            )
                            del stage_tiles[st]
    nc.compile()
    return nc


def plan_and_build(src_idx, dst_idx, e_att):
    src_idx = np.asarray(src_idx, dtype=np.int64)
    dst_idx = np.asarray(dst_idx, dtype=np.int64)
    att_flat = np.asarray(e_att, dtype=np.float16).reshape(-1)
    tok = (src_idx // 2).astype(np.int16)
    half = (src_idx & 1).astype(np.int64)
    return _plan(dst_idx, tok, half, att_flat)


def kernel(src_emb, e_att, src_idx, dst_idx):
    from concourse.bass_utils import run_bass_kernel_spmd

    src_emb = np.asarray(src_emb, dtype=np.float32)
    pl = plan_and_build(src_idx, dst_idx, e_att)

    key = (pl["NB"], pl["chunks"])
    if key not in _cache:
        _cache.clear()
        _cache[key] = _build_nc(pl["NB"], pl["chunks"])
    nc = _cache[key]

    embP = np.zeros((NPAIR * 2, D), dtype=np.float16)
    embP[:N_SRC] = src_emb.astype(np.float16)
    embP = np.ascontiguousarray(embP.reshape(NPAIR, P))

    NB = pl["NB"]
    in_maps = []
    for c in range(N_CORES):
        # atab layout: [128 (slot in block), NB*64] fp16
        at = np.ascontiguousarray(
            pl["a3"][c].reshape(NB, P, 64).transpose(1, 0, 2).reshape(P, NB * 64)
        )
        in_maps.append(
            {
                "embP": embP,
                "idxT": np.ascontiguousarray(_wrap_idx(pl["idx2"][c].reshape(-1))),
                "atab": at,
            }
        )
    kwargs = {}
    if TRACE:
        kwargs = {"trace": True, "tmpdir": TRACE_DIR}
    res = run_bass_kernel_spmd(nc, in_maps, core_ids=list(range(N_CORES)), **kwargs)
    global LAST_EXEC_NS
    LAST_EXEC_NS = res.exec_time_ns

    out_full = np.zeros((N_DST, D), dtype=np.float32)
    G = pl["G"]
    node_at = pl["node_at"]  # [ng_pad, 32]
    for c in range(N_CORES):
        ids = node_at[c * G : (c + 1) * G].reshape(-1)   # supertile-row order
        valid = ids >= 0
        out_full[ids[valid]] = res.results[c]["out"][valid]
    return out_full
